# revision 1
# baseline (speedup 1.0000x reference)
"""Trainium2 Bass kernel for nn_DependencyEncoder (shift-reduce tree-LSTM encoder).

Structure exploited: the transition sequence strictly alternates
shift (odd steps) / reduce (even steps), so the parser stack depth
oscillates 2->3->2...  Consequently:
  stack[0] is frozen at token0 forever,
  stack[1] holds a single running composed value v,
  each shifted token is consumed by the immediately following reduce.
The whole module collapses to 63 iterations of:
  shift  t=2k : tracking LSTM on x=[tok_k, v, tok_0]
  reduce t=2k+1: tracking LSTM on x=[tok_{k+1}, tok_k, v], then
                 head = tok_k (left-arc) or v (right-arc),
                 gates = W_{left/right} @ head + W_c @ th,  v <- (h_j, c_j)
Output = v_h after the last pair.  (Validated vs reference in numpy.)

Sharding: pure batch data-parallel, 384 rows -> 8 cores x 48 rows.
Per core (batch B=48):
  - activations are the PE stationary operand ("fm" feature-major layout,
    feature chunks 128/128/44 on partitions, batch on free), the big weight
    matrices are the streamed operand -> no weight reloads ever;
  - all matmul operands are float32r: full-rate fp32 PE mode (measured
    ~1.6e-4 rel err per matmul; exact fp32 streams at 1/4 rate).  float32r
    matmuls use the whole array (no column tiling), so gates are computed in
    one partition group, batch-major [48, 1500];
  - tanh is computed as 2*sigmoid(2x)-1 (u/g weight rows pre-scaled by 2 on
    the host) so every transcendental is a single Sigmoid activation op;
  - left/right arc selection is done by masking the stationary operand
    (h_l = token*mask_l, h_r = v*mask_r) and accumulating both W_left and
    W_right streams into the same PSUM.
"""
import numpy as np

import concourse.bacc as bacc
import concourse.bass as bass
import concourse.mybir as mybir
import concourse.tile as tile
from concourse.alu_op_type import AluOpType as alu
from concourse.bass import AP
from concourse.bass_utils import run_bass_kernel_spmd
from concourse.masks import make_identity

AF = mybir.ActivationFunctionType
f32 = mybir.dt.float32
f32r = mybir.dt.float32r

B_FULL, L, H, TD = 384, 64, 300, 64
NCORES = 8
B = B_FULL // NCORES          # 48 rows per core
K = int(__import__("os").environ.get("KERNEL_PAIRS", L - 1))   # 63 pairs
CH = [(0, 128), (128, 128), (256, 44)]   # feature chunks (offset, size)
NGATE = 5 * H                 # 1500 composition gate columns
NCHUNKS = ((0, 512), (512, 1024), (1024, NGATE))


# --------------------------------------------------------------------------
# host-side weight preparation
# --------------------------------------------------------------------------
def _comp_rhs(Wmat):
    """[5H, Kin] -> streamed rhs [Kin, 1500] with gate blocks reordered to
    (i, fh, fc, u, o) so sigmoid(o) can run as a separate parallel op;
    u-gate rows pre-scaled by 2 for the tanh->sigmoid trick."""
    Wg = Wmat.astype(np.float32).reshape(5, H, -1)
    # (i, o, fh, fc, u) -> (fh, fc, i, u, o): forget gates first so their
    # sigmoid + sum run while the rest of the gates stream
    Wg = np.concatenate([Wg[2:3], Wg[3:4], Wg[0:1], 2.0 * Wg[4:5], Wg[1:2]],
                        axis=0)
    return np.ascontiguousarray(Wg.reshape(5 * H, -1).T)


def _chunkify(Wr):
    """[Kin<=300, C] -> [128, 3, C] zero-padded feature chunks."""
    out = np.zeros((128, 3, Wr.shape[1]), np.float32)
    for c, (off, sz) in enumerate(CH):
        out[:sz, c, :] = Wr[off:off + sz, :]
    return out


def _prep_host(inputs):
    W_c = np.asarray(inputs["W_c"], np.float32)
    Uh_w = np.asarray(inputs["Uh_w"], np.float32)
    Ul_w = np.asarray(inputs["Ul_w"], np.float32)
    Ur_w = np.asarray(inputs["Ur_w"], np.float32)
    W_ih = np.asarray(inputs["W_ih"], np.float32)
    W_hh = np.asarray(inputs["W_hh"], np.float32)

    wl = _chunkify(_comp_rhs(Uh_w + Ul_w))        # [128, 3, 1500]
    wr = _chunkify(_comp_rhs(Uh_w + Ur_w))
    wcc = _comp_rhs(W_c)                          # [64, 1500]

    # tracking: torch gate order (i,f,g,o) -> (i,f,o,g), g rows *2
    perm = np.concatenate([np.arange(0, 64), np.arange(64, 128),
                           np.arange(192, 256), np.arange(128, 192)])
    scl = np.ones(256, np.float32)
    scl[192:] = 2.0
    Wih_r = W_ih[perm, :] * scl[:, None]          # [256, 900]
    Whh_r = (W_hh[perm, :] * scl[:, None]).T.copy()   # [64, 256]
    wtrk = np.zeros((128, 9, 256), np.float32)
    for s in range(3):
        for c, (off, sz) in enumerate(CH):
            wtrk[:sz, s * 3 + c, :] = Wih_r[:, s * H + off: s * H + off + sz].T
    return dict(wl=wl, wr=wr, wcc=np.ascontiguousarray(wcc),
                wtrk=wtrk, whh=np.ascontiguousarray(Whh_r))


# --------------------------------------------------------------------------
# device program
# --------------------------------------------------------------------------
_CACHED_NC = None


def _build_nc():
    nc = bacc.Bacc("TRN2", target_bir_lowering=False)
    tokh_d = nc.dram_tensor("tokh", [128, L, 3, B], f32r, kind="ExternalInput")
    tokc_d = nc.dram_tensor("tokc", [L, B, H], f32, kind="ExternalInput")
    wl_d = nc.dram_tensor("wl", [128, 3, NGATE], f32r, kind="ExternalInput")
    wr_d = nc.dram_tensor("wr", [128, 3, NGATE], f32r, kind="ExternalInput")
    wcc_d = nc.dram_tensor("wcc", [TD, NGATE], f32r, kind="ExternalInput")
    wtrk_d = nc.dram_tensor("wtrk", [128, 9, 256], f32r, kind="ExternalInput")
    whh_d = nc.dram_tensor("whh", [TD, 256], f32r, kind="ExternalInput")
    mlr_d = nc.dram_tensor("mlr", [K, B], f32, kind="ExternalInput")
    mrr_d = nc.dram_tensor("mrr", [K, B], f32, kind="ExternalInput")
    mld_d = nc.dram_tensor("mld", [B, K], f32, kind="ExternalInput")
    mrd_d = nc.dram_tensor("mrd", [B, K], f32, kind="ExternalInput")
    th0t_d = nc.dram_tensor("th0t", [TD, B], f32r, kind="ExternalInput")
    tc0_d = nc.dram_tensor("tc0", [B, TD], f32, kind="ExternalInput")
    out_d = nc.dram_tensor("out", [3, 128, B], f32r, kind="ExternalOutput")

    with tile.TileContext(nc) as tc_:
        with (
            tc_.tile_pool(name="sg", bufs=1) as sg,
            tc_.tile_pool(name="rot", bufs=3) as rot,
            tc_.tile_pool(name="tkc", bufs=3) as tkc,
            tc_.tile_pool(name="st", bufs=6) as st,
            tc_.tile_pool(name="psA", bufs=3, space="PSUM") as psA,
            tc_.tile_pool(name="psT", bufs=2, space="PSUM") as psT,
            tc_.tile_pool(name="psC", bufs=1, space="PSUM") as psC,
        ):
            # ---------------- resident tiles ----------------
            tokh = sg.tile([128, L, 3, B], f32r)    # feature-major tokens (h)
            wl = sg.tile([128, 3, NGATE], f32r)
            wr = sg.tile([128, 3, NGATE], f32r)
            wcc = sg.tile([TD, NGATE], f32r)
            wtrk = sg.tile([128, 9, 256], f32r)
            whh = sg.tile([TD, 256], f32r)
            mlr = sg.tile([128, K, B], f32)         # left mask bcast over partitions
            mrr = sg.tile([128, K, B], f32)
            mld = sg.tile([B, K], f32)              # per-partition masks (batch rows)
            mrd = sg.tile([B, K], f32)
            th0t = sg.tile([TD, B], f32r)
            tc0s = sg.tile([B, TD], f32)
            ident = sg.tile([128, 128], f32)

            make_identity(nc, ident[:])

            # all layout transforms were done host-side: plain copies only.
            # tokens arrive in 8 groups so pair-0 compute starts early
            for gvii in range(8):
                gl = gvii * (L // 8)
                nc.sync.dma_start(tokh[:, gl:gl + L // 8, :, :],
                                  tokh_d[:, gl:gl + L // 8, :, :])
            nc.sync.dma_start(wl[:], wl_d[:])
            nc.sync.dma_start(wr[:], wr_d[:])
            nc.sync.dma_start(wcc[:], wcc_d[:])
            nc.sync.dma_start(wtrk[:], wtrk_d[:])
            nc.sync.dma_start(whh[:], whh_d[:])
            for dst, srcd in ((mlr, mlr_d), (mrr, mrr_d)):
                bsrc = AP(tensor=srcd, offset=0, ap=[[0, 128], [B, K], [1, B]])
                nc.sync.dma_start(dst[:], bsrc)
            nc.sync.dma_start(mld[:], mld_d[:])
            nc.sync.dma_start(mrd[:], mrd_d[:])
            nc.sync.dma_start(th0t[:], th0t_d[:])
            nc.sync.dma_start(tc0s[:], tc0_d[:])

            # composition psum: persistent, 3 banks
            cp = psC.tile([B, 1536], f32)

            # float32r matmuls: full-rate fp32 PE mode
            mm = nc.tensor.matmul

            def tok_ap(c, l):
                return tokh[:CH[c][1], l, c, :]

            def track_mms(bufs_l, s1_l, s2_l, thT_in=None):
                """Emit the 9 x-stream matmuls (+U if thT_in) for one step.
                Order (buf, s2, s1): the s1 operand is the freshest value, so
                it goes last in the PE's in-order queue."""
                ps = psA.tile([B, 256], f32, tag="trk")
                first = True
                for s, srcl in ((0, bufs_l), (2, s2_l), (1, s1_l)):
                    for c in range(3):
                        sz = CH[c][1]
                        mm(ps[:], srcl[c], wtrk[:sz, s * 3 + c, :],
                           start=first, stop=False)
                        first = False
                if thT_in is not None:
                    mm(ps[:], thT_in[:], whh[:], start=False, stop=True)
                return ps

            def track_tail(ps, tc_in):
                """Sigmoid + LSTM cell + transposed next-th for one step."""
                sa = rot.tile([B, 256], f32, tag="sa")
                nc.scalar.activation(sa[:], ps[:], AF.Sigmoid)
                d3 = st.tile([B, TD], f32, tag="d3")
                nc.vector.tensor_tensor(d3[:], sa[:, 64:128], tc_in[:], alu.mult)
                d1 = st.tile([B, TD], f32, tag="d1")
                nc.gpsimd.tensor_tensor(d1[:], sa[:, 0:64], sa[:, 192:256], alu.mult)
                d2 = st.tile([B, TD], f32, tag="d2")
                nc.vector.scalar_tensor_tensor(d2[:], d1[:], 2.0, sa[:, 0:64],
                                               alu.mult, alu.subtract)
                tc_o = st.tile([B, TD], f32, tag="tc")
                nc.vector.tensor_tensor(tc_o[:], d3[:], d2[:], alu.add)
                ptc = psT.tile([128, B], f32, tag="ptr")
                nc.tensor.transpose(ptc[0:TD, :], tc_o[:], ident[0:B, 0:B])
                pso = psT.tile([128, B], f32, tag="ptr")
                nc.tensor.transpose(pso[0:TD, :], sa[:, 128:192], ident[0:B, 0:B])
                sT = st.tile([TD, B], f32, tag="sT")
                nc.scalar.activation(sT[:], ptc[0:TD, :], AF.Sigmoid, scale=2.0)
                soT = st.tile([TD, B], f32, tag="soT")
                nc.vector.tensor_copy(soT[:], pso[0:TD, :])
                pp = st.tile([TD, B], f32, tag="pp")
                nc.vector.tensor_tensor(pp[:], sT[:], soT[:], alu.mult)
                thT_o = st.tile([TD, B], f32r, tag="thT")
                nc.vector.scalar_tensor_tensor(thT_o[:], pp[:], 2.0, soT[:],
                                               alu.mult, alu.subtract)
                return thT_o, tc_o

            thT_prev = th0t
            tc_prev = tc0s
            vh_prev = None          # fm chunks of running value v (h)
            vc_prev = None          # batch-major v (c) [48, 300]

            for k in range(K):
                mlr_k = mlr[:, k, :]
                mrr_k = mrr[:, k, :]
                mld_k = mld[:, k:k + 1]
                mrd_k = mrd[:, k:k + 1]

                if k == 0:
                    vh_l = [tok_ap(c, 0) for c in range(3)]
                else:
                    vh_l = [vh_prev[:CH[c][1], c, :] for c in range(3)]

                # ---- c tokens for this pair: streamed from DRAM
                tokc_t = tkc.tile([B, H], f32, tag="tokc")
                nc.sync.dma_start(tokc_t[:], tokc_d[k, :, :])

                # ---- head tiles for composition
                hl = rot.tile([128, 3, B], f32r, tag="hl")
                hr = rot.tile([128, 3, B], f32r, tag="hr")
                for c, (off, sz) in enumerate(CH):
                    nc.gpsimd.tensor_tensor(hl[:sz, c, :], tok_ap(c, k),
                                            mlr_k[:sz, :], alu.mult)
                    nc.gpsimd.tensor_tensor(hr[:sz, c, :], vh_l[c],
                                            mrr_k[:sz, :], alu.mult)
                ch1 = rot.tile([B, H], f32, tag="ch1")
                nc.gpsimd.tensor_scalar(ch1[:], tokc_t[:], mld_k, None, alu.mult)
                ch = rot.tile([B, H], f32, tag="ch")
                if k == 0:
                    nc.vector.scalar_tensor_tensor(ch[:], tokc_t[:], mrd_k,
                                                   ch1[:], alu.mult, alu.add)
                else:
                    nc.vector.scalar_tensor_tensor(ch[:], vc_prev[:], mrd_k,
                                                   ch1[:], alu.mult, alu.add)

                # ---- matmul emission order = PE in-order queue.
                # track-a streams first (its U uses thT_prev: no stall), then
                # track-b x-streams, then its U (waits thT_a), then the fat
                # composition streams overlapping the track-b tail.
                toks_k = [tok_ap(c, k) for c in range(3)]
                toks_k1 = [tok_ap(c, k + 1) for c in range(3)]
                toks_0 = [tok_ap(c, 0) for c in range(3)]
                ps_a = track_mms(toks_k, vh_l, toks_0, thT_in=thT_prev)
                thT_a, tc_a = track_tail(ps_a, tc_prev)
                ps_b = track_mms(toks_k1, toks_k, vh_l)
                mm(ps_b[:], thT_a[:], whh[:], start=False, stop=True)
                thT_b, tc_b = track_tail(ps_b, tc_a)

                # ---- composition matmuls (heads; W_c accumulated after track)
                for nlo, nhi in NCHUNKS:
                    for c, (off, sz) in enumerate(CH):
                        mm(cp[:, nlo:nhi], hl[:sz, c, :], wl[:sz, c, nlo:nhi],
                           start=(c == 0), stop=False)
                    for c, (off, sz) in enumerate(CH):
                        mm(cp[:, nlo:nhi], hr[:sz, c, :], wr[:sz, c, nlo:nhi],
                           start=False, stop=False)

                # ---- W_c stream into composition psum
                for nlo, nhi in NCHUNKS:
                    mm(cp[:, nlo:nhi], thT_b[:], wcc[:, nlo:nhi],
                       start=False, stop=True)

                # ---- composition elementwise, batch-major [48, 300] slices
                sc = rot.tile([B, NGATE], f32, tag="sc")
                nc.scalar.activation(sc[:, 0:600], cp[:, 0:600], AF.Sigmoid)
                nc.scalar.activation(sc[:, 600:1200], cp[:, 600:1200],
                                     AF.Sigmoid)
                nc.scalar.activation(sc[:, 1200:1500], cp[:, 1200:1500],
                                     AF.Sigmoid)
                SCfh = sc[:, 0:300]
                SCfc = sc[:, 300:600]
                SCi = sc[:, 600:900]
                SCu = sc[:, 900:1200]
                SCo = sc[:, 1200:1500]
                t2 = rot.tile([B, H], f32, tag="t2")
                nc.gpsimd.tensor_tensor(t2[:], SCfh, SCfc, alu.add)
                t3 = rot.tile([B, H], f32, tag="t3")
                nc.gpsimd.tensor_tensor(t3[:], t2[:], ch[:], alu.mult)
                pu = rot.tile([B, H], f32, tag="pu")
                nc.vector.tensor_tensor(pu[:], SCi, SCu, alu.mult)
                xu = rot.tile([B, H], f32, tag="xu")
                nc.vector.scalar_tensor_tensor(xu[:], pu[:], 2.0, SCi,
                                               alu.mult, alu.subtract)
                c_j = rot.tile([B, H], f32, tag="vc")
                nc.vector.tensor_tensor(c_j[:], xu[:], t3[:], alu.add)
                scj = rot.tile([B, H], f32, tag="scj")
                nc.scalar.activation(scj[:], c_j[:], AF.Sigmoid, scale=2.0)
                # h_j, its transpose and the feature-major copy are emitted
                # per feature chunk so the next pair's v-dependent matmuls can
                # start as soon as their chunk lands
                h_j = rot.tile([B, H], f32, tag="hj")
                vh = rot.tile([128, 3, B], f32r, tag="vh")
                copy_eng = (nc.vector.tensor_copy,
                            lambda o, i: nc.scalar.activation(o, i, AF.Copy),
                            nc.vector.tensor_copy)
                for c, (off, sz) in enumerate(CH):
                    qq = rot.tile([B, 128], f32, tag="qq")
                    nc.vector.tensor_tensor(qq[:, :sz], scj[:, off:off + sz],
                                            SCo[:, off:off + sz], alu.mult)
                    nc.vector.scalar_tensor_tensor(
                        h_j[:, off:off + sz], qq[:, :sz], 2.0,
                        SCo[:, off:off + sz], alu.mult, alu.subtract)
                    pc = psT.tile([128, B], f32, tag="ptr")
                    nc.tensor.transpose(pc[0:sz, :], h_j[:, off:off + sz],
                                        ident[0:B, 0:B])
                    copy_eng[c](vh[:sz, c, :], pc[0:sz, :])

                vh_prev, vc_prev = vh, c_j
                thT_prev, tc_prev = thT_b, tc_b

            # ---- output: v_h in feature-major chunk layout [3, 128, B]
            for c in range(3):
                nc.sync.dma_start(out_d[c, :, :], vh_prev[:, c, :])

    nc.compile()
    return nc


def _get_nc():
    global _CACHED_NC
    if _CACHED_NC is None:
        _CACHED_NC = _build_nc()
    return _CACHED_NC


def make_in_maps(inputs):
    """Build the 8 per-core input maps from the full-problem inputs."""
    seq = np.asarray(inputs["sequence"], np.float32)
    tr = np.asarray(inputs["transitions"])
    th0 = np.asarray(inputs["th0"], np.float32)
    tc0 = np.asarray(inputs["tc0"], np.float32)
    wts = _prep_host(inputs)

    in_maps = []
    for i in range(NCORES):
        s = slice(i * B, (i + 1) * B)
        sq = seq[s]                                  # [B, L, 600]
        # feature-major h tokens [128, L, 3, B]
        tokh = np.zeros((128, L, 3, B), np.float32)
        for c, (off, sz) in enumerate(CH):
            tokh[:sz, :, c] = sq[:, :, off:off + sz].transpose(2, 1, 0)
        # c tokens, token-major for per-pair streaming
        tokc = np.ascontiguousarray(sq[:, :, H:].transpose(1, 0, 2))  # [L,B,H]

        is_left = (tr[s, 1::2].T == 2).astype(np.float32)[:K]   # [K, B]
        in_maps.append(dict(
            tokh=tokh, tokc=tokc,
            wl=wts["wl"], wr=wts["wr"], wcc=wts["wcc"],
            wtrk=wts["wtrk"], whh=wts["whh"],
            mlr=np.ascontiguousarray(is_left),
            mrr=np.ascontiguousarray(1.0 - is_left),
            mld=np.ascontiguousarray(is_left.T),
            mrd=np.ascontiguousarray(1.0 - is_left.T),
            th0t=np.ascontiguousarray(th0[s].T),
            tc0=np.ascontiguousarray(tc0[s]),
        ))
    return in_maps


def assemble_out(res_list):
    """Per-core [3, 128, B] chunk outputs -> [B_full, 300] float32."""
    outs = []
    for r in res_list:
        arr = r["out"]                       # [3, 128, B]
        o = np.empty((B, H), np.float32)
        for c, (off, sz) in enumerate(CH):
            o[:, off:off + sz] = arr[c, :sz, :].T
        outs.append(o)
    return np.concatenate(outs, axis=0)


def kernel(**inputs) -> np.ndarray:
    nc = _get_nc()
    in_maps = make_in_maps(inputs)
    res = run_bass_kernel_spmd(nc, in_maps, core_ids=list(range(NCORES)))
    return assemble_out(res.results)

